# revision 5
# baseline (speedup 1.0000x reference)
"""GraphSAGE-style GNN layer on 8 Trainium2 NeuronCores.

out = relu(W @ concat([features[nodes], mean(features[neigh_idx], 1)], 1).T)

Strategy: replicate the feature table (host-cast to bf16) on all 8 cores;
data-parallel over the 16384-node batch (2048 nodes/core; 22528 gathered
rows/core). The classic per-row indirect-DMA path costs ~1.7us of Pool/SWDGE
time per 128 rows (the old 382us baseline was 81% Pool-bound), so gathers
run through the MoE bulk-DMA instructions instead:

L1: the core's 22528 row requests are host-sorted by 32768-row table
    segment; one dma_gather per segment (31 total, int16 within-segment
    indices, idx-0 dummy padding to a static count) streams every row into
    a segment-sorted SBUF staging area (stream pos i -> partition i%128,
    512B stripe i//128).
L2: 8 SBUF-source transposing dma_gathers (int16 stream-position indices,
    host-computed) re-gather the staged rows in (tile, slot, node) order,
    emitting feature-major [128, 2, 2816] tiles directly (no PE transposes).
Then DVE tree-sums the 10 neighbor slots down to 2, PE runs the bf16
[256,512] weight matmul (neighbor-mean folded into W2/10 on host), ScalarE
applies ReLU, and the bf16 output is upcast on host.
"""
import numpy as np

N_CORES = 8
NUM_NODES = 1_000_000
F = 256
E = 256
B = 16384
NSAMP = 10
SLOTS = 1 + NSAMP
P = 128
B_LOCAL = B // N_CORES          # 2048
TILES = B_LOCAL // P            # 16
GROUP = 2                       # node-tiles per L2 gather / matmul group
NGROUPS = TILES // GROUP        # 8
L2N = GROUP * SLOTS * P         # 2816 rows per L2 gather
NREQ = B_LOCAL * SLOTS          # 22528 rows per core
SEG = 32768                     # int16-indexable table segment
NSEG = (NUM_NODES + SEG - 1) // SEG   # 31

_cache = {}


def _build(nidx1):
    import concourse.bacc as bacc
    import concourse.mybir as mybir
    import concourse.tile as tile

    bf16 = mybir.dt.bfloat16
    f32 = mybir.dt.float32
    add = mybir.AluOpType.add
    bypass = mybir.AluOpType.bypass

    stripes1 = nidx1 // P           # staging stripes per segment
    nstripes = NSEG * stripes1      # total staging stripes
    c1 = nidx1 // 16                # idx cols per segment
    c2 = L2N // 16                  # idx cols per group

    nc = bacc.Bacc("TRN2", target_bir_lowering=False, debug=False)
    feats = nc.dram_tensor("features", [NUM_NODES, F], bf16,
                           kind="ExternalInput")
    wt = nc.dram_tensor("wt", [2 * F, E], bf16, kind="ExternalInput")
    ix1 = nc.dram_tensor("ix1", [P, NSEG * c1], mybir.dt.int16,
                         kind="ExternalInput")
    ix2 = nc.dram_tensor("ix2", [P, NGROUPS * c2], mybir.dt.int16,
                         kind="ExternalInput")
    out = nc.dram_tensor("out", [E, B_LOCAL], bf16, kind="ExternalOutput")

    with tile.TileContext(nc) as tc:
        with (
            tc.tile_pool(name="const", bufs=1) as constp,
            tc.tile_pool(name="gat", bufs=3) as gatp,
            tc.tile_pool(name="tmp", bufs=2) as tmpp,
            tc.tile_pool(name="outs", bufs=3) as outsp,
            tc.tile_pool(name="psm", bufs=8, space="PSUM") as psmp,
        ):
            wtile = constp.tile([P, 4 * E], bf16)
            nc.sync.dma_start(
                out=wtile[:].rearrange("k (c e) -> k c e", c=4),
                in_=wt.ap().rearrange("(c k) e -> k c e", k=P),
            )
            ix1t = constp.tile([P, NSEG * c1], mybir.dt.int16)
            nc.sync.dma_start(out=ix1t[:], in_=ix1.ap())
            ix2t = constp.tile([P, NGROUPS * c2], mybir.dt.int16)
            nc.sync.dma_start(out=ix2t[:], in_=ix2.ap())

            stag = constp.tile([P, nstripes * F], bf16)
            for m in range(NSEG):
                lo = m * SEG
                hi = min((m + 1) * SEG, NUM_NODES)
                nc.gpsimd.dma_gather(
                    out_ap=stag[:, m * stripes1 * F:(m + 1) * stripes1 * F]
                    .rearrange("p (a f) -> p a f", a=stripes1),
                    in_ap=feats.ap()[lo:hi, :],
                    idxs_ap=ix1t[:, m * c1:(m + 1) * c1],
                    num_idxs=nidx1,
                    num_idxs_reg=nidx1,
                    elem_size=F,
                    # 896 idx = 57 descs/engine, under the 64-desc packet
                    # concat limit; larger pads must not concatenate
                    single_packet=(nidx1 <= 896),
                )

            for g in range(NGROUPS):
                gt = gatp.tile([P, 2 * L2N], bf16, tag="gt")
                nc.gpsimd.dma_gather(
                    out_ap=gt[:].rearrange("p (c i) -> p c i", c=2),
                    in_ap=stag[:],
                    idxs_ap=ix2t[:, g * c2:(g + 1) * c2],
                    num_idxs=L2N,
                    num_idxs_reg=L2N,
                    elem_size=F,
                    transpose=True,
                    sbuf_tokens_per_rank=P,
                    sbuf_free_dim_per_rank=2 * F,
                    # >64 descriptors per engine exceeds the SDMA packet
                    # concat limit; one packet per descriptor
                    single_packet=False,
                )
                # gt free layout: (cb=2c+b: 4, s: 11, i: 128) where c = feat
                # chunk, b = tile-in-group; slot 0 = self, 1..10 = neighbors
                V = gt[:].rearrange("p (cb s i) -> p cb s i", cb=4, s=SLOTS)
                t1 = tmpp.tile([P, 4 * 4 * P], bf16, tag="t1")
                V1 = t1[:].rearrange("p (cb s i) -> p cb s i", cb=4, s=4)
                nc.vector.scalar_tensor_tensor(
                    out=V1, in0=V[:, :, 1:5, :], scalar=0.0,
                    in1=V[:, :, 5:9, :], op0=bypass, op1=add)
                t2 = tmpp.tile([P, 4 * 2 * P], bf16, tag="t2")
                V2 = t2[:].rearrange("p (cb s i) -> p cb s i", cb=4, s=2)
                nc.vector.scalar_tensor_tensor(
                    out=V2, in0=V1[:, :, 0:2, :], scalar=0.0,
                    in1=V1[:, :, 2:4, :], op0=bypass, op1=add)
                t3 = tmpp.tile([P, 4 * 2 * P], bf16, tag="t3")
                V3 = t3[:].rearrange("p (cb s i) -> p cb s i", cb=4, s=2)
                nc.vector.scalar_tensor_tensor(
                    out=V3, in0=V2, scalar=0.0,
                    in1=V[:, :, 9:11, :], op0=bypass, op1=add)

                for ec in range(2):
                    o = outsp.tile([P, GROUP * P], bf16, tag="o")
                    for b in range(GROUP):
                        pm = psmp.tile([P, P], f32, tag="pm")
                        for c in range(2):
                            nc.tensor.matmul(
                                out=pm[:],
                                lhsT=wtile[:, c * E + ec * P:
                                           c * E + ec * P + P],
                                rhs=V[:, 2 * c + b, 0, :],
                                start=(c == 0), stop=False)
                        for c in range(2):
                            for pr in range(2):
                                nc.tensor.matmul(
                                    out=pm[:],
                                    lhsT=wtile[:, (2 + c) * E + ec * P:
                                               (2 + c) * E + ec * P + P],
                                    rhs=V3[:, 2 * c + b, pr, :],
                                    start=False, stop=(c == 1 and pr == 1))
                        nc.scalar.activation(o[:, b * P:(b + 1) * P], pm[:],
                                             mybir.ActivationFunctionType.Relu)
                    nc.sync.dma_start(
                        out=out.ap()[ec * P:(ec + 1) * P,
                                     g * GROUP * P:(g + 1) * GROUP * P],
                        in_=o[:])
    nc.compile()
    return nc


def _get_nc(nidx1):
    if nidx1 not in _cache:
        _cache[nidx1] = _build(nidx1)
    return _cache[nidx1]


def _wrap16(blocks):
    """[n_blocks, L] int16 -> [128, n_blocks*(L//16)]: per block, idx list
    position i -> (partition i%16, col i//16), replicated across the 8
    Q7-core partition groups."""
    n, L = blocks.shape
    t = blocks.reshape(n, L // 16, 16).transpose(0, 2, 1)  # [n, 16, L//16]
    t = np.concatenate(list(t), axis=1)                    # [16, n*L//16]
    return np.ascontiguousarray(np.tile(t, (8, 1)))


def run(features, W, nodes, neigh_idx, trace=False):
    import ml_dtypes
    from concourse.bass_utils import run_bass_kernel_spmd

    bf16 = ml_dtypes.bfloat16
    features = np.ascontiguousarray(np.asarray(features)).astype(bf16)
    W = np.asarray(W, dtype=np.float32)
    nodes = np.asarray(nodes).astype(np.int64)
    neigh = np.asarray(neigh_idx).astype(np.int64)

    wt = np.ascontiguousarray(
        np.concatenate([W[:, :F].T, W[:, F:].T / NSAMP], axis=0)).astype(bf16)

    percore = []
    maxcnt = 0
    for c in range(N_CORES):
        sl = slice(c * B_LOCAL, (c + 1) * B_LOCAL)
        gx = np.concatenate([nodes[sl, None], neigh[sl]], axis=1)  # [2048,11]
        rows = gx.reshape(TILES, P, SLOTS).transpose(0, 2, 1).reshape(-1)
        seg = rows >> 15
        within = (rows & (SEG - 1)).astype(np.int16)
        order = np.argsort(seg, kind="stable")
        cnt = np.bincount(seg, minlength=NSEG)
        maxcnt = max(maxcnt, int(cnt.max()))
        percore.append((seg, within, order, cnt))

    nidx1 = max(896, -(-maxcnt // P) * P)
    nc = _get_nc(nidx1)

    in_maps = []
    for c in range(N_CORES):
        seg, within, order, cnt = percore[c]
        starts = np.zeros(NSEG, np.int64)
        starts[1:] = np.cumsum(cnt)[:-1]
        ranks = np.arange(NREQ) - starts[seg[order]]
        pos = np.empty(NREQ, np.int64)
        pos[order] = seg[order] * nidx1 + ranks
        l1 = np.zeros((NSEG, nidx1), np.int16)   # idx-0 dummy padding
        w_sorted = within[order]
        for m in range(NSEG):
            l1[m, :cnt[m]] = w_sorted[starts[m]:starts[m] + cnt[m]]
        ix1 = _wrap16(l1)
        ix2 = _wrap16(pos.astype(np.int16).reshape(NGROUPS, L2N))
        in_maps.append({"features": features, "wt": wt,
                        "ix1": ix1, "ix2": ix2})

    res = run_bass_kernel_spmd(nc, in_maps,
                               core_ids=list(range(N_CORES)), trace=trace)
    out = np.concatenate([r["out"] for r in res.results], axis=1)
    return out.astype(np.float32), res


def kernel(features, W, nodes, neigh_idx):
    out, _ = run(features, W, nodes, neigh_idx)
    return out
